# revision 20
# baseline (speedup 1.0000x reference)
"""Trainium2 Bass kernel for nn_MixtureCogrammar.

Computation (reference):
    attn  = softmax(morphosyn @ W_affix)                    [B, V]
    affix = attn @ affix_vocab.reshape(V, D*N)              [B, D, N]
    wC    = cumsum_n( sum_{ijk} a_i b_j f_k softmax(pivot_logits[i,j,:,k,:]) )
    out   = stem + wC * (affix - stem)

Distribution: the D axis (free dim of the dominant [B,V]x[V,D*N] matmul)
is sharded over the 8 cores (D_local = 32 per core). Every core computes
the full attention (cheap), the pivot/wC path is batch-sharded with an
AllGather, and affix_vocab / stem / out are D-sharded so each core
touches 1/8 of the heavy tensors.

Per-core device program:
  - mixture weights a (x) b (x) f computed on device from alpha/beta/phi
  - attn computed per 128-batch chunk, transposed on the PE into [V, B]
  - big matmul accumulates 4 K-chunks (bf16) plus a (-I) @ stem matmul
    (float32r) into PSUM, so PSUM holds delta = affix - stem
  - DVE blends: out = stem + wC * delta (2 tensor-tensor ops per tile)
"""

import os
import sys

import numpy as np

for _p in ("/opt/trn_rl_repo",):
    if os.path.isdir(_p) and _p not in sys.path:
        sys.path.append(_p)

import concourse.bass as bass  # noqa: E402
import concourse.tile as tile  # noqa: E402
from concourse import bacc, mybir  # noqa: E402
from concourse.bass import ts  # noqa: E402
from concourse.bass_utils import run_bass_kernel_spmd  # noqa: E402
from concourse.masks import make_identity  # noqa: E402

import ml_dtypes  # noqa: E402

B, D, N, DM, V = 1024, 256, 256, 128, 512
NCORES = 8
DLOC = D // NCORES          # 32 d-values per core
BCH = B // 128              # 8 batch chunks
DN = DLOC * N               # 8192 free elems per core
HALF = DN // 2              # 4096 per round
NT = HALF // 512            # 8 psum tiles per (chunk, round)
DHALF = DLOC // 2           # 16 d-values per round

F32 = mybir.dt.float32
F32R = mybir.dt.float32r
BF16 = mybir.dt.bfloat16
EXP = mybir.ActivationFunctionType.Exp
ALU = mybir.AluOpType

# knobs
VOCAB_BF16 = True    # host-cast affix_vocab to bf16 (halves its DMA)
STEM_BF16 = True     # host-cast stem to bf16 (halves stem DMA, adds ~1e-3 err)
OUT_BF16 = True      # write output as bf16, upcast on host

LAST_RESULT = None   # BassKernelResults of the last run (exec_time_ns etc.)

_CACHE = {}


def _build():
    key = (VOCAB_BF16, STEM_BF16, OUT_BF16)
    if key in _CACHE:
        return _CACHE[key]

    # stem is declared float32r (same bytes as fp32) so the (-I) @ stem
    # matmul runs at full PE speed; DVE ops bitcast it back to fp32.
    stem_dt = BF16 if STEM_BF16 else F32R
    vocab_dt = BF16 if VOCAB_BF16 else F32
    out_dt = BF16 if OUT_BF16 else F32

    nc = bacc.Bacc("TRN2", target_bir_lowering=False, debug=False,
                   num_devices=NCORES)

    stem_d = nc.dram_tensor("stem", [B, DLOC, N], stem_dt, kind="ExternalInput").ap()
    vocab_d = nc.dram_tensor("vocab", [V, DLOC, N], vocab_dt, kind="ExternalInput").ap()
    mor_d = nc.dram_tensor("morpho", [B, DM], F32, kind="ExternalInput").ap()
    waff_d = nc.dram_tensor("waffix", [DM, V], F32, kind="ExternalInput").ap()
    pv_d = nc.dram_tensor("pivot", [2, 2, 128, 5, N], F32, kind="ExternalInput").ap()
    abf_d = nc.dram_tensor("abf", [1, 9], F32, kind="ExternalInput").ap()
    nident_d = nc.dram_tensor("nident", [128, 128], stem_dt, kind="ExternalInput").ap()
    out_d = nc.dram_tensor("out", [B, DLOC, N], out_dt, kind="ExternalOutput").ap()

    from contextlib import ExitStack

    with tile.TileContext(nc) as tc, ExitStack() as ctx:
        const = ctx.enter_context(tc.tile_pool(name="const", bufs=1))

        ident = const.tile([128, 128], F32)
        make_identity(nc, ident[:, :])
        neg_ident = const.tile([128, 128], stem_dt)
        nc.sync.dma_start(neg_ident[:, :], nident_d[:, :])

        attnT = const.tile([128, 4, B], BF16)      # [v_part, vc, b]
        wc_sb = const.tile([128, BCH, N], BF16)    # [b_part, cb, n]
        w_bcast = const.tile([128, 20], F32)
        wsb = const.tile([128, V], F32)            # W_affix resident
        vocab_sb = const.tile([128, 4, DN], vocab_dt)  # [v_part, vc, (d n)]

        # ---------- phase A: mixture weights ----------
        small = ctx.enter_context(tc.tile_pool(name="small", bufs=1))
        abf = small.tile([1, 9], F32)
        nc.sync.dma_start(abf[0:1, :], abf_d[:, :])
        nc.sync.dma_start(wsb[:, :], waff_d[:, :])
        eabf = small.tile([1, 9], F32)
        sums = small.tile([1, 3], F32)
        nc.scalar.activation(eabf[0:1, 0:2], abf[0:1, 0:2], EXP, accum_out=sums[0:1, 0:1])
        nc.scalar.activation(eabf[0:1, 2:4], abf[0:1, 2:4], EXP, accum_out=sums[0:1, 1:2])
        nc.scalar.activation(eabf[0:1, 4:9], abf[0:1, 4:9], EXP, accum_out=sums[0:1, 2:3])
        rsum = small.tile([1, 3], F32)
        nc.vector.reciprocal(rsum[0:1, :], sums[0:1, :])
        t4 = small.tile([1, 4], F32)
        nc.vector.tensor_mul(
            t4[0:1, :].rearrange("p (i j) -> p i j", i=2),
            eabf[0:1, 0:2].rearrange("p (i j) -> p i j", j=1).to_broadcast((1, 2, 2)),
            eabf[0:1, 2:4].rearrange("p (i j) -> p i j", i=1).to_broadcast((1, 2, 2)),
        )
        t20 = small.tile([1, 20], F32)
        nc.vector.tensor_mul(
            t20[0:1, :].rearrange("p (g k) -> p g k", g=4),
            t4[0:1, :].rearrange("p (g k) -> p g k", k=1).to_broadcast((1, 4, 5)),
            eabf[0:1, 4:9].rearrange("p (g k) -> p g k", g=1).to_broadcast((1, 4, 5)),
        )
        rr = small.tile([1, 1], F32)
        nc.vector.tensor_mul(rr[0:1, :], rsum[0:1, 0:1], rsum[0:1, 1:2])
        rrr = small.tile([1, 1], F32)
        nc.vector.tensor_mul(rrr[0:1, :], rr[0:1, :], rsum[0:1, 2:3])
        w20 = small.tile([1, 20], F32)
        nc.vector.tensor_scalar_mul(w20[0:1, :], t20[0:1, :], rrr[0:1, 0:1])
        nc.gpsimd.partition_broadcast(w_bcast[:, :], w20[0:1, :])

        # ---------- phase C: pivots (this core's batch chunk) ----------
        with tc.tile_pool(name="pv", bufs=1) as pvp:
            pv = pvp.tile([128, 4, 5, N], F32)
            for ij in range(4):
                i, j = divmod(ij, 2)
                nc.sync.dma_start(pv[:, ij, :, :], pv_d[i, j, :, :, :])
            pvE = pvp.tile([128, 20, N], F32)
            sP = pvp.tile([128, 20], F32)
            for g in range(20):
                nc.scalar.activation(pvE[:, g, :], pv[:, g // 5, g % 5, :], EXP,
                                     accum_out=sP[:, g:g + 1])
            rP = pvp.tile([128, 20], F32)
            nc.vector.reciprocal(rP[:, :], sP[:, :])
            rPw = pvp.tile([128, 20], F32)
            nc.vector.tensor_mul(rPw[:, :], rP[:, :], w_bcast[:, :])
            accA = pvp.tile([128, N], F32)
            accB = pvp.tile([128, N], F32)
            nc.vector.tensor_scalar_mul(accA[:, :], pvE[:, 0, :], rPw[:, 0:1])
            cur, nxt = accA, accB
            for g in range(1, 20):
                nc.vector.scalar_tensor_tensor(
                    out=nxt[:, :], in0=pvE[:, g, :], scalar=rPw[:, g:g + 1],
                    in1=cur[:, :], op0=ALU.mult, op1=ALU.add,
                )
                cur, nxt = nxt, cur
            wCl = pvp.tile([128, N], BF16)
            nc.vector.tensor_tensor_scan(
                wCl[:, :], data0=cur[:, :], data1=cur[:, :], initial=0.0,
                op0=ALU.add, op1=ALU.bypass,
            )
            dram = ctx.enter_context(tc.tile_pool(name="dram", bufs=1, space="DRAM"))
            wc_in = dram.tile([128, N], BF16)
            wc_out = nc.dram_tensor("wc_gath", [B, N], BF16,
                                    addr_space="Shared").ap()
            nc.sync.dma_start(wc_in[:, :], wCl[:, :])
            nc.gpsimd.collective_compute(
                "AllGather", ALU.bypass,
                replica_groups=[list(range(NCORES))],
                ins=[wc_in[:, :].opt()], outs=[wc_out[:, :].opt()],
            )
            nc.sync.dma_start(
                wc_sb[:, :, :],
                wc_out[:, :].rearrange("(c p) n -> p c n", p=128),
            )

        # vocab loads: emitted after the pivot path so the small latency-
        # critical DMAs get queue priority; r0 halves first.
        for r in range(2):
            for vc in range(4):
                nc.sync.dma_start(
                    vocab_sb[:, vc, r * HALF:(r + 1) * HALF],
                    vocab_d[ts(vc, 128), ts(r, DHALF), :].rearrange("p d n -> p (d n)"),
                )

        # ---------- phase B: attention for all batches ----------
        with tc.tile_pool(name="attn", bufs=2) as bp, \
             tc.tile_pool(name="psB", bufs=2, space="PSUM") as psB, \
             tc.tile_pool(name="psT", bufs=2, space="PSUM") as psT:
            for cb in range(BCH):
                mor = bp.tile([128, DM], F32)
                nc.sync.dma_start(mor[:, :], mor_d[ts(cb, 128), :])
                morT_ps = psB.tile([128, DM], F32, tag="morT_ps")
                nc.tensor.transpose(morT_ps[:, :], mor[:, :], ident[:, :])
                morT = bp.tile([128, DM], F32)
                nc.scalar.copy(morT[:, :], morT_ps[:, :])
                lg_ps = psB.tile([128, V], F32, tag="lg_ps")
                nc.tensor.matmul(lg_ps[:, :], lhsT=morT[:, :], rhs=wsb[:, :],
                                 start=True, stop=True)
                E = bp.tile([128, V], F32)
                sE = bp.tile([128, 1], F32)
                nc.scalar.activation(E[:, :], lg_ps[:, :], EXP, accum_out=sE[:, :])
                rE = bp.tile([128, 1], F32)
                nc.vector.reciprocal(rE[:, :], sE[:, :])
                attn = bp.tile([128, V], F32)
                nc.scalar.mul(attn[:, :], E[:, :], rE[:, 0:1])
                for vc in range(4):
                    tp = psT.tile([128, 128], F32)
                    nc.tensor.transpose(tp[:, :], attn[:, ts(vc, 128)], ident[:, :])
                    nc.scalar.copy(attnT[:, vc, ts(cb, 128)], tp[:, :])

        # ---------- phase D: main loop ----------
        # stem bufs deep enough that the identity-matmuls of groups emitted
        # while the AllGather is in flight are not blocked on stem slot reuse
        stp = ctx.enter_context(tc.tile_pool(name="stem", bufs=5))
        otp = ctx.enter_context(tc.tile_pool(name="outp", bufs=4))
        prp = ctx.enter_context(tc.tile_pool(name="prod", bufs=3))
        psD = ctx.enter_context(tc.tile_pool(name="psD", bufs=2, space="PSUM"))

        PSW = 2048              # one psum tile = 4 banks = 4 matmul groups
        NH = HALF // PSW        # 2 psum tiles per (cb, round)
        # Every delta tile is spilled PSUM -> SBUF bf16 on the (otherwise
        # idle) ScalarE. This (a) frees PSUM fast so the PE is never
        # backpressured while the wC AllGather is in flight, and (b) makes
        # both blend tensor-tensor ops all-SBUF 2-byte operands, enabling
        # the DVE 2x perf mode.
        dlp = ctx.enter_context(tc.tile_pool(name="delta", bufs=10))

        for r in range(2):
            for cb in range(BCH):
                stem_t = stp.tile([128, HALF], stem_dt)
                nc.sync.dma_start(
                    stem_t[:, :],
                    stem_d[ts(cb, 128), ts(r, DHALF), :].rearrange("p d n -> p (d n)"),
                )
                for h in range(NH):
                    ps = psD.tile([128, PSW], F32)
                    for t in range(PSW // 512):
                        col = h * PSW + t * 512
                        for vc in range(4):
                            nc.tensor.matmul(
                                ps[:, ts(t, 512)],
                                lhsT=attnT[:, vc:vc + 1, ts(cb, 128)],
                                rhs=vocab_sb[:, vc, r * HALF + col: r * HALF + col + 512],
                                start=(vc == 0), stop=False,
                            )
                        nc.tensor.matmul(
                            ps[:, ts(t, 512)],
                            lhsT=neg_ident[:, :],
                            rhs=stem_t[:, bass.ds(col, 512)],
                            start=False, stop=True,
                        )
                    delta_t = dlp.tile([128, PSW], BF16)
                    nc.scalar.copy(delta_t[:, :], ps[:, :])
                    prod = prp.tile([128, PSW], BF16)
                    nc.vector.tensor_mul(
                        prod[:, :].rearrange("p (a n) -> p a n", n=N),
                        delta_t[:, :].rearrange("p (a n) -> p a n", n=N),
                        wc_sb[:, cb:cb + 1, :].to_broadcast((128, PSW // N, N)),
                    )
                    out_t = otp.tile([128, PSW], out_dt)
                    nc.vector.tensor_add(out_t[:, :], prod[:, :],
                                         stem_t[:, ts(h, PSW)])
                    nc.sync.dma_start(
                        out_d[ts(cb, 128), bass.ds(r * DHALF + h * (PSW // N), PSW // N), :]
                        .rearrange("p d n -> p (d n)"),
                        out_t[:, :],
                    )

    nc.compile()
    _CACHE[key] = nc
    return nc


def kernel(stem_form, morphosyn, pivot_logits, W_affix, affix_vocab,
           alpha, beta, phi, max_len):
    global LAST_RESULT
    stem_form = np.ascontiguousarray(np.asarray(stem_form, dtype=np.float32))
    morphosyn = np.ascontiguousarray(np.asarray(morphosyn, dtype=np.float32))
    pivot_logits = np.ascontiguousarray(np.asarray(pivot_logits, dtype=np.float32))
    W_affix = np.ascontiguousarray(np.asarray(W_affix, dtype=np.float32))
    affix_vocab = np.ascontiguousarray(np.asarray(affix_vocab, dtype=np.float32))
    abf = np.concatenate([
        np.asarray(alpha, np.float32).ravel(),
        np.asarray(beta, np.float32).ravel(),
        np.asarray(phi, np.float32).ravel(),
    ]).reshape(1, 9)

    nc = _build()

    stem_np = stem_form.astype(ml_dtypes.bfloat16) if STEM_BF16 else stem_form
    vocab_np = affix_vocab.astype(ml_dtypes.bfloat16) if VOCAB_BF16 else affix_vocab
    neg_eye = -np.eye(128, dtype=np.float32)
    if STEM_BF16:
        neg_eye = neg_eye.astype(ml_dtypes.bfloat16)

    in_maps = []
    for c in range(NCORES):
        dlo, dhi = c * DLOC, (c + 1) * DLOC
        in_maps.append({
            "stem": np.ascontiguousarray(stem_np[:, dlo:dhi, :]),
            "vocab": np.ascontiguousarray(vocab_np[:, dlo:dhi, :]),
            "morpho": morphosyn,
            "waffix": W_affix,
            "pivot": np.ascontiguousarray(pivot_logits[:, :, c * 128:(c + 1) * 128, :, :]),
            "abf": abf,
            "nident": neg_eye,
        })

    LAST_RESULT = run_bass_kernel_spmd(nc, in_maps, core_ids=list(range(NCORES)))
    outs = [LAST_RESULT.results[c]["out"] for c in range(NCORES)]
    out = np.concatenate([o.astype(np.float32) for o in outs], axis=1)
    return np.ascontiguousarray(out)


# revision 22
# speedup vs baseline: 1.1261x; 1.1261x over previous
"""Trainium2 Bass kernel for nn_MixtureCogrammar.

Computation (reference):
    attn  = softmax(morphosyn @ W_affix)                    [B, V]
    affix = attn @ affix_vocab.reshape(V, D*N)              [B, D, N]
    wC    = cumsum_n( sum_{ijk} a_i b_j f_k softmax(pivot_logits[i,j,:,k,:]) )
    out   = stem + wC * (affix - stem)

Distribution: the D axis (free dim of the dominant [B,V]x[V,D*N] matmul)
is sharded over the 8 cores (D_local = 32 per core). Every core computes
the full attention (cheap), the pivot/wC path is batch-sharded with an
AllGather, and affix_vocab / stem / out are D-sharded so each core
touches 1/8 of the heavy tensors.

Per-core device program:
  - mixture weights a (x) b (x) f computed on device from alpha/beta/phi
  - attn computed per 128-batch chunk, transposed on the PE into [V, B]
  - big matmul accumulates 4 K-chunks (bf16) plus a (-I) @ stem matmul
    (float32r) into PSUM, so PSUM holds delta = affix - stem
  - DVE blends: out = stem + wC * delta (2 tensor-tensor ops per tile)
"""

import os
import sys

import numpy as np

for _p in ("/opt/trn_rl_repo",):
    if os.path.isdir(_p) and _p not in sys.path:
        sys.path.append(_p)

import concourse.bass as bass  # noqa: E402
import concourse.tile as tile  # noqa: E402
from concourse import bacc, mybir  # noqa: E402
from concourse.bass import ts  # noqa: E402
from concourse.bass_utils import run_bass_kernel_spmd  # noqa: E402
from concourse.masks import make_identity  # noqa: E402

import ml_dtypes  # noqa: E402

B, D, N, DM, V = 1024, 256, 256, 128, 512
NCORES = 8
DLOC = D // NCORES          # 32 d-values per core
BCH = B // 128              # 8 batch chunks
DN = DLOC * N               # 8192 free elems per core
HALF = DN // 2              # 4096 per round
NT = HALF // 512            # 8 psum tiles per (chunk, round)
DHALF = DLOC // 2           # 16 d-values per round

F32 = mybir.dt.float32
F32R = mybir.dt.float32r
BF16 = mybir.dt.bfloat16
EXP = mybir.ActivationFunctionType.Exp
ALU = mybir.AluOpType

# knobs
VOCAB_BF16 = True    # host-cast affix_vocab to bf16 (halves its DMA)
STEM_BF16 = True     # host-cast stem to bf16 (halves stem DMA, adds ~1e-3 err)
OUT_BF16 = True      # write output as bf16, upcast on host

LAST_RESULT = None   # BassKernelResults of the last run (exec_time_ns etc.)

_CACHE = {}


def _build():
    key = (VOCAB_BF16, STEM_BF16, OUT_BF16)
    if key in _CACHE:
        return _CACHE[key]

    stem_dt = BF16 if STEM_BF16 else F32
    vocab_dt = BF16 if VOCAB_BF16 else F32
    out_dt = BF16 if OUT_BF16 else F32

    nc = bacc.Bacc("TRN2", target_bir_lowering=False, debug=False,
                   num_devices=NCORES)

    stem_d = nc.dram_tensor("stem", [B, DLOC, N], stem_dt, kind="ExternalInput").ap()
    vocab_d = nc.dram_tensor("vocab", [V, DLOC, N], vocab_dt, kind="ExternalInput").ap()
    mor_d = nc.dram_tensor("morpho", [B, DM], F32, kind="ExternalInput").ap()
    waff_d = nc.dram_tensor("waffix", [DM, V], F32, kind="ExternalInput").ap()
    pv_d = nc.dram_tensor("pivot", [2, 2, 128, 5, N], F32, kind="ExternalInput").ap()
    abf_d = nc.dram_tensor("abf", [1, 9], F32, kind="ExternalInput").ap()
    out_d = nc.dram_tensor("out", [B, DLOC, N], out_dt, kind="ExternalOutput").ap()

    from contextlib import ExitStack

    with tile.TileContext(nc) as tc, ExitStack() as ctx:
        const = ctx.enter_context(tc.tile_pool(name="const", bufs=1))

        ident = const.tile([128, 128], F32)
        make_identity(nc, ident[:, :])

        attnT = const.tile([128, 4, B], BF16)      # [v_part, vc, b]
        wc_sb = const.tile([128, BCH, N], BF16)    # [b_part, cb, n]
        w_bcast = const.tile([128, 20], F32)
        wsb = const.tile([128, V], F32)            # W_affix resident
        vocab_sb = const.tile([128, 4, DN], vocab_dt)  # [v_part, vc, (d n)]

        # ---------- phase A: mixture weights ----------
        small = ctx.enter_context(tc.tile_pool(name="small", bufs=1))
        abf = small.tile([1, 9], F32)
        nc.sync.dma_start(abf[0:1, :], abf_d[:, :])
        nc.sync.dma_start(wsb[:, :], waff_d[:, :])
        eabf = small.tile([1, 9], F32)
        sums = small.tile([1, 3], F32)
        nc.scalar.activation(eabf[0:1, 0:2], abf[0:1, 0:2], EXP, accum_out=sums[0:1, 0:1])
        nc.scalar.activation(eabf[0:1, 2:4], abf[0:1, 2:4], EXP, accum_out=sums[0:1, 1:2])
        nc.scalar.activation(eabf[0:1, 4:9], abf[0:1, 4:9], EXP, accum_out=sums[0:1, 2:3])
        rsum = small.tile([1, 3], F32)
        nc.vector.reciprocal(rsum[0:1, :], sums[0:1, :])
        t4 = small.tile([1, 4], F32)
        nc.vector.tensor_mul(
            t4[0:1, :].rearrange("p (i j) -> p i j", i=2),
            eabf[0:1, 0:2].rearrange("p (i j) -> p i j", j=1).to_broadcast((1, 2, 2)),
            eabf[0:1, 2:4].rearrange("p (i j) -> p i j", i=1).to_broadcast((1, 2, 2)),
        )
        t20 = small.tile([1, 20], F32)
        nc.vector.tensor_mul(
            t20[0:1, :].rearrange("p (g k) -> p g k", g=4),
            t4[0:1, :].rearrange("p (g k) -> p g k", k=1).to_broadcast((1, 4, 5)),
            eabf[0:1, 4:9].rearrange("p (g k) -> p g k", g=1).to_broadcast((1, 4, 5)),
        )
        rr = small.tile([1, 1], F32)
        nc.vector.tensor_mul(rr[0:1, :], rsum[0:1, 0:1], rsum[0:1, 1:2])
        rrr = small.tile([1, 1], F32)
        nc.vector.tensor_mul(rrr[0:1, :], rr[0:1, :], rsum[0:1, 2:3])
        w20 = small.tile([1, 20], F32)
        nc.vector.tensor_scalar_mul(w20[0:1, :], t20[0:1, :], rrr[0:1, 0:1])
        nc.gpsimd.partition_broadcast(w_bcast[:, :], w20[0:1, :])

        # ---------- phase C: pivots (this core's batch chunk) ----------
        with tc.tile_pool(name="pv", bufs=1) as pvp:
            pv = pvp.tile([128, 4, 5, N], F32)
            for ij in range(4):
                i, j = divmod(ij, 2)
                nc.sync.dma_start(pv[:, ij, :, :], pv_d[i, j, :, :, :])
            pvE = pvp.tile([128, 20, N], F32)
            sP = pvp.tile([128, 20], F32)
            for g in range(20):
                nc.scalar.activation(pvE[:, g, :], pv[:, g // 5, g % 5, :], EXP,
                                     accum_out=sP[:, g:g + 1])
            rP = pvp.tile([128, 20], F32)
            nc.vector.reciprocal(rP[:, :], sP[:, :])
            rPw = pvp.tile([128, 20], F32)
            nc.vector.tensor_mul(rPw[:, :], rP[:, :], w_bcast[:, :])
            accA = pvp.tile([128, N], F32)
            accB = pvp.tile([128, N], F32)
            nc.vector.tensor_scalar_mul(accA[:, :], pvE[:, 0, :], rPw[:, 0:1])
            cur, nxt = accA, accB
            for g in range(1, 20):
                nc.vector.scalar_tensor_tensor(
                    out=nxt[:, :], in0=pvE[:, g, :], scalar=rPw[:, g:g + 1],
                    in1=cur[:, :], op0=ALU.mult, op1=ALU.add,
                )
                cur, nxt = nxt, cur
            wCl = pvp.tile([128, N], BF16)
            nc.vector.tensor_tensor_scan(
                wCl[:, :], data0=cur[:, :], data1=cur[:, :], initial=0.0,
                op0=ALU.add, op1=ALU.bypass,
            )
            dram = ctx.enter_context(tc.tile_pool(name="dram", bufs=1, space="DRAM"))
            wc_in = dram.tile([128, N], BF16)
            wc_out = nc.dram_tensor("wc_gath", [B, N], BF16,
                                    addr_space="Shared").ap()
            nc.sync.dma_start(wc_in[:, :], wCl[:, :])
            nc.gpsimd.collective_compute(
                "AllGather", ALU.bypass,
                replica_groups=[list(range(NCORES))],
                ins=[wc_in[:, :].opt()], outs=[wc_out[:, :].opt()],
            )
            nc.sync.dma_start(
                wc_sb[:, :, :],
                wc_out[:, :].rearrange("(c p) n -> p c n", p=128),
            )

        # vocab loads: emitted after the pivot path so the small latency-
        # critical DMAs get queue priority; r0 halves first.
        for r in range(2):
            for vc in range(4):
                nc.sync.dma_start(
                    vocab_sb[:, vc, r * HALF:(r + 1) * HALF],
                    vocab_d[ts(vc, 128), ts(r, DHALF), :].rearrange("p d n -> p (d n)"),
                )

        # ---------- phase B: attention for all batches ----------
        with tc.tile_pool(name="attn", bufs=2) as bp, \
             tc.tile_pool(name="psB", bufs=2, space="PSUM") as psB, \
             tc.tile_pool(name="psT", bufs=2, space="PSUM") as psT:
            for cb in range(BCH):
                mor = bp.tile([128, DM], F32)
                nc.sync.dma_start(mor[:, :], mor_d[ts(cb, 128), :])
                morT_ps = psB.tile([128, DM], F32, tag="morT_ps")
                nc.tensor.transpose(morT_ps[:, :], mor[:, :], ident[:, :])
                morT = bp.tile([128, DM], F32)
                nc.scalar.copy(morT[:, :], morT_ps[:, :])
                lg_ps = psB.tile([128, V], F32, tag="lg_ps")
                nc.tensor.matmul(lg_ps[:, :], lhsT=morT[:, :], rhs=wsb[:, :],
                                 start=True, stop=True)
                E = bp.tile([128, V], F32)
                sE = bp.tile([128, 1], F32)
                nc.scalar.activation(E[:, :], lg_ps[:, :], EXP, accum_out=sE[:, :])
                rE = bp.tile([128, 1], F32)
                nc.vector.reciprocal(rE[:, :], sE[:, :])
                attn = bp.tile([128, V], F32)
                nc.scalar.mul(attn[:, :], E[:, :], rE[:, 0:1])
                for vc in range(4):
                    tp = psT.tile([128, 128], F32)
                    nc.tensor.transpose(tp[:, :], attn[:, ts(vc, 128)], ident[:, :])
                    nc.scalar.copy(attnT[:, vc, ts(cb, 128)], tp[:, :])

        # ---------- phase D: main loop ----------
        # stem bufs deep enough that the identity-matmuls of groups emitted
        # while the AllGather is in flight are not blocked on stem slot reuse
        stp = ctx.enter_context(tc.tile_pool(name="stem", bufs=5))
        otp = ctx.enter_context(tc.tile_pool(name="outp", bufs=4))
        prp = ctx.enter_context(tc.tile_pool(name="prod", bufs=3))
        psD = ctx.enter_context(tc.tile_pool(name="psD", bufs=2, space="PSUM"))

        PSW = 2048              # one psum tile = 4 banks = 4 matmul groups
        NH = HALF // PSW        # 2 psum tiles per (cb, round)
        # Every delta tile is spilled PSUM -> SBUF bf16 on the (otherwise
        # idle) ScalarE. This (a) frees PSUM fast so the PE is never
        # backpressured while the wC AllGather is in flight, and (b) makes
        # both blend tensor-tensor ops all-SBUF 2-byte operands, enabling
        # the DVE 2x perf mode.
        dlp = ctx.enter_context(tc.tile_pool(name="delta", bufs=10))

        for r in range(2):
            for cb in range(BCH):
                stem_t = stp.tile([128, HALF], stem_dt)
                nc.sync.dma_start(
                    stem_t[:, :],
                    stem_d[ts(cb, 128), ts(r, DHALF), :].rearrange("p d n -> p (d n)"),
                )
                for h in range(NH):
                    ps = psD.tile([128, PSW], F32)
                    for t in range(PSW // 512):
                        col = h * PSW + t * 512
                        for vc in range(4):
                            nc.tensor.matmul(
                                ps[:, ts(t, 512)],
                                lhsT=attnT[:, vc:vc + 1, ts(cb, 128)],
                                rhs=vocab_sb[:, vc, r * HALF + col: r * HALF + col + 512],
                                start=(vc == 0), stop=(vc == 3),
                            )
                    # delta = affix - stem; also drains PSUM so the PE is
                    # never blocked on the wC AllGather
                    delta_t = dlp.tile([128, PSW], BF16)
                    nc.vector.tensor_sub(delta_t[:, :], ps[:, :],
                                         stem_t[:, ts(h, PSW)])
                    prod = prp.tile([128, PSW], BF16)
                    nc.vector.tensor_mul(
                        prod[:, :].rearrange("p (a n) -> p a n", n=N),
                        delta_t[:, :].rearrange("p (a n) -> p a n", n=N),
                        wc_sb[:, cb:cb + 1, :].to_broadcast((128, PSW // N, N)),
                    )
                    out_t = otp.tile([128, PSW], out_dt)
                    nc.vector.tensor_add(out_t[:, :], prod[:, :],
                                         stem_t[:, ts(h, PSW)])
                    nc.sync.dma_start(
                        out_d[ts(cb, 128), bass.ds(r * DHALF + h * (PSW // N), PSW // N), :]
                        .rearrange("p d n -> p (d n)"),
                        out_t[:, :],
                    )

    nc.compile()
    _CACHE[key] = nc
    return nc


def kernel(stem_form, morphosyn, pivot_logits, W_affix, affix_vocab,
           alpha, beta, phi, max_len):
    global LAST_RESULT
    stem_form = np.ascontiguousarray(np.asarray(stem_form, dtype=np.float32))
    morphosyn = np.ascontiguousarray(np.asarray(morphosyn, dtype=np.float32))
    pivot_logits = np.ascontiguousarray(np.asarray(pivot_logits, dtype=np.float32))
    W_affix = np.ascontiguousarray(np.asarray(W_affix, dtype=np.float32))
    affix_vocab = np.ascontiguousarray(np.asarray(affix_vocab, dtype=np.float32))
    abf = np.concatenate([
        np.asarray(alpha, np.float32).ravel(),
        np.asarray(beta, np.float32).ravel(),
        np.asarray(phi, np.float32).ravel(),
    ]).reshape(1, 9)

    nc = _build()

    stem_np = stem_form.astype(ml_dtypes.bfloat16) if STEM_BF16 else stem_form
    vocab_np = affix_vocab.astype(ml_dtypes.bfloat16) if VOCAB_BF16 else affix_vocab

    in_maps = []
    for c in range(NCORES):
        dlo, dhi = c * DLOC, (c + 1) * DLOC
        in_maps.append({
            "stem": np.ascontiguousarray(stem_np[:, dlo:dhi, :]),
            "vocab": np.ascontiguousarray(vocab_np[:, dlo:dhi, :]),
            "morpho": morphosyn,
            "waffix": W_affix,
            "pivot": np.ascontiguousarray(pivot_logits[:, :, c * 128:(c + 1) * 128, :, :]),
            "abf": abf,
        })

    LAST_RESULT = run_bass_kernel_spmd(nc, in_maps, core_ids=list(range(NCORES)))
    outs = [LAST_RESULT.results[c]["out"] for c in range(NCORES)]
    out = np.concatenate([o.astype(np.float32) for o in outs], axis=1)
    return np.ascontiguousarray(out)


# revision 26
# speedup vs baseline: 1.2298x; 1.0920x over previous
"""Trainium2 Bass kernel for nn_MixtureCogrammar.

Computation (reference):
    attn  = softmax(morphosyn @ W_affix)                    [B, V]
    affix = attn @ affix_vocab.reshape(V, D*N)              [B, D, N]
    wC    = cumsum_n( sum_{ijk} a_i b_j f_k softmax(pivot_logits[i,j,:,k,:]) )
    out   = stem + wC * (affix - stem)

Distribution: the D axis (free dim of the dominant [B,V]x[V,D*N] matmul)
is sharded over the 8 cores (D_local = 32 per core). Every core computes
the full attention (cheap), the pivot/wC path is batch-sharded with an
AllGather, and affix_vocab / stem / out are D-sharded so each core
touches 1/8 of the heavy tensors.

Per-core device program:
  - mixture weights a (x) b (x) f computed on device from alpha/beta/phi
  - attn computed per 128-batch chunk, transposed on the PE into [V, B]
  - big matmul accumulates 4 K-chunks (bf16) plus a (-I) @ stem matmul
    (float32r) into PSUM, so PSUM holds delta = affix - stem
  - DVE blends: out = stem + wC * delta (2 tensor-tensor ops per tile)
"""

import os
import sys

import numpy as np

for _p in ("/opt/trn_rl_repo",):
    if os.path.isdir(_p) and _p not in sys.path:
        sys.path.append(_p)

import concourse.bass as bass  # noqa: E402
import concourse.tile as tile  # noqa: E402
from concourse import bacc, mybir  # noqa: E402
from concourse.bass import ts  # noqa: E402
from concourse.bass_utils import run_bass_kernel_spmd  # noqa: E402
from concourse.masks import make_identity  # noqa: E402

import ml_dtypes  # noqa: E402

B, D, N, DM, V = 1024, 256, 256, 128, 512
NCORES = 8
DLOC = D // NCORES          # 32 d-values per core
BCH = B // 128              # 8 batch chunks
DN = DLOC * N               # 8192 free elems per core
HALF = DN // 2              # 4096 per round
NT = HALF // 512            # 8 psum tiles per (chunk, round)
DHALF = DLOC // 2           # 16 d-values per round

F32 = mybir.dt.float32
F32R = mybir.dt.float32r
BF16 = mybir.dt.bfloat16
EXP = mybir.ActivationFunctionType.Exp
ALU = mybir.AluOpType

# knobs
VOCAB_BF16 = True    # host-cast affix_vocab to bf16 (halves its DMA)
STEM_BF16 = True     # host-cast stem to bf16 (halves stem DMA, adds ~1e-3 err)
OUT_BF16 = True      # write output as bf16, upcast on host

LAST_RESULT = None   # BassKernelResults of the last run (exec_time_ns etc.)

_CACHE = {}


def _build():
    key = (VOCAB_BF16, STEM_BF16, OUT_BF16)
    if key in _CACHE:
        return _CACHE[key]

    stem_dt = BF16 if STEM_BF16 else F32
    vocab_dt = BF16 if VOCAB_BF16 else F32
    out_dt = BF16 if OUT_BF16 else F32

    nc = bacc.Bacc("TRN2", target_bir_lowering=False, debug=False,
                   num_devices=NCORES)

    stem_d = nc.dram_tensor("stem", [B, DLOC, N], stem_dt, kind="ExternalInput").ap()
    vocab_d = nc.dram_tensor("vocab", [V, DLOC, N], vocab_dt, kind="ExternalInput").ap()
    mor_d = nc.dram_tensor("morpho", [B, DM], F32, kind="ExternalInput").ap()
    waff_d = nc.dram_tensor("waffix", [DM, V], F32, kind="ExternalInput").ap()
    pv_d = nc.dram_tensor("pivot", [2, 2, 128, 5, N], F32, kind="ExternalInput").ap()
    abf_d = nc.dram_tensor("abf", [1, 9], F32, kind="ExternalInput").ap()
    out_d = nc.dram_tensor("out", [B, DLOC, N], out_dt, kind="ExternalOutput").ap()

    from contextlib import ExitStack

    with tile.TileContext(nc) as tc, ExitStack() as ctx:
        const = ctx.enter_context(tc.tile_pool(name="const", bufs=1))

        ident = const.tile([128, 128], F32)
        make_identity(nc, ident[:, :])

        attnT = const.tile([128, 4, B], BF16)      # [v_part, vc, b]
        wc_sb = const.tile([128, BCH, N], BF16)    # [b_part, cb, n]
        w_bcast = const.tile([128, 20], F32)
        wsb = const.tile([128, V], F32)            # W_affix resident
        mor_all = const.tile([128, BCH, DM], F32)  # morphosyn, all chunks
        vocab_sb = const.tile([128, 4, DN], vocab_dt)  # [v_part, vc, (d n)]

        # ---------- phase A: mixture weights ----------
        small = ctx.enter_context(tc.tile_pool(name="small", bufs=1))
        abf = small.tile([1, 9], F32)
        nc.sync.dma_start(abf[0:1, :], abf_d[:, :])
        nc.sync.dma_start(wsb[:, :], waff_d[:, :])
        for cb in range(BCH):
            nc.sync.dma_start(mor_all[:, cb, :], mor_d[ts(cb, 128), :])
        eabf = small.tile([1, 9], F32)
        sums = small.tile([1, 3], F32)
        nc.scalar.activation(eabf[0:1, 0:2], abf[0:1, 0:2], EXP, accum_out=sums[0:1, 0:1])
        nc.scalar.activation(eabf[0:1, 2:4], abf[0:1, 2:4], EXP, accum_out=sums[0:1, 1:2])
        nc.scalar.activation(eabf[0:1, 4:9], abf[0:1, 4:9], EXP, accum_out=sums[0:1, 2:3])
        rsum = small.tile([1, 3], F32)
        nc.vector.reciprocal(rsum[0:1, :], sums[0:1, :])
        t4 = small.tile([1, 4], F32)
        nc.vector.tensor_mul(
            t4[0:1, :].rearrange("p (i j) -> p i j", i=2),
            eabf[0:1, 0:2].rearrange("p (i j) -> p i j", j=1).to_broadcast((1, 2, 2)),
            eabf[0:1, 2:4].rearrange("p (i j) -> p i j", i=1).to_broadcast((1, 2, 2)),
        )
        t20 = small.tile([1, 20], F32)
        nc.vector.tensor_mul(
            t20[0:1, :].rearrange("p (g k) -> p g k", g=4),
            t4[0:1, :].rearrange("p (g k) -> p g k", k=1).to_broadcast((1, 4, 5)),
            eabf[0:1, 4:9].rearrange("p (g k) -> p g k", g=1).to_broadcast((1, 4, 5)),
        )
        rr = small.tile([1, 1], F32)
        nc.vector.tensor_mul(rr[0:1, :], rsum[0:1, 0:1], rsum[0:1, 1:2])
        rrr = small.tile([1, 1], F32)
        nc.vector.tensor_mul(rrr[0:1, :], rr[0:1, :], rsum[0:1, 2:3])
        w20 = small.tile([1, 20], F32)
        nc.vector.tensor_scalar_mul(w20[0:1, :], t20[0:1, :], rrr[0:1, 0:1])
        nc.gpsimd.partition_broadcast(w_bcast[:, :], w20[0:1, :])

        # ---------- phase C: pivots (this core's batch chunk) ----------
        with tc.tile_pool(name="pv", bufs=1) as pvp:
            pv = pvp.tile([128, 4, 5, N], F32)
            for ij in range(4):
                i, j = divmod(ij, 2)
                nc.sync.dma_start(pv[:, ij, :, :], pv_d[i, j, :, :, :])
            pvE = pvp.tile([128, 20, N], F32)
            sP = pvp.tile([128, 20], F32)
            for g in range(20):
                nc.scalar.activation(pvE[:, g, :], pv[:, g // 5, g % 5, :], EXP,
                                     accum_out=sP[:, g:g + 1])
            rP = pvp.tile([128, 20], F32)
            nc.vector.reciprocal(rP[:, :], sP[:, :])
            rPw = pvp.tile([128, 20], F32)
            nc.vector.tensor_mul(rPw[:, :], rP[:, :], w_bcast[:, :])
            accA = pvp.tile([128, N], F32)
            accB = pvp.tile([128, N], F32)
            nc.vector.tensor_scalar_mul(accA[:, :], pvE[:, 0, :], rPw[:, 0:1])
            cur, nxt = accA, accB
            for g in range(1, 20):
                nc.vector.scalar_tensor_tensor(
                    out=nxt[:, :], in0=pvE[:, g, :], scalar=rPw[:, g:g + 1],
                    in1=cur[:, :], op0=ALU.mult, op1=ALU.add,
                )
                cur, nxt = nxt, cur
            wCl = pvp.tile([128, N], BF16)
            nc.vector.tensor_tensor_scan(
                wCl[:, :], data0=cur[:, :], data1=cur[:, :], initial=0.0,
                op0=ALU.add, op1=ALU.bypass,
            )
            dram = ctx.enter_context(tc.tile_pool(name="dram", bufs=1, space="DRAM"))
            wc_in = dram.tile([128, N], BF16)
            wc_out = nc.dram_tensor("wc_gath", [B, N], BF16,
                                    addr_space="Shared").ap()
            nc.sync.dma_start(wc_in[:, :], wCl[:, :])
            nc.gpsimd.collective_compute(
                "AllGather", ALU.bypass,
                replica_groups=[list(range(NCORES))],
                ins=[wc_in[:, :].opt()], outs=[wc_out[:, :].opt()],
            )
            nc.sync.dma_start(
                wc_sb[:, :, :],
                wc_out[:, :].rearrange("(c p) n -> p c n", p=128),
            )

        # vocab loads: emitted after the pivot path so the small latency-
        # critical DMAs get queue priority; r0 halves first.
        for r in range(2):
            for vc in range(4):
                nc.sync.dma_start(
                    vocab_sb[:, vc, r * HALF:(r + 1) * HALF],
                    vocab_d[ts(vc, 128), ts(r, DHALF), :].rearrange("p d n -> p (d n)"),
                )

        # ---------- phase B: attention for all batches ----------
        with tc.tile_pool(name="attn", bufs=2) as bp, \
             tc.tile_pool(name="psB", bufs=2, space="PSUM") as psB, \
             tc.tile_pool(name="psT", bufs=2, space="PSUM") as psT:
            for cb in range(BCH):
                morT_ps = psB.tile([128, DM], F32, tag="morT_ps")
                nc.tensor.transpose(morT_ps[:, :], mor_all[:, cb, :], ident[:, :])
                morT = bp.tile([128, DM], F32)
                nc.scalar.copy(morT[:, :], morT_ps[:, :])
                lg_ps = psB.tile([128, V], F32, tag="lg_ps")
                nc.tensor.matmul(lg_ps[:, :], lhsT=morT[:, :], rhs=wsb[:, :],
                                 start=True, stop=True)
                E = bp.tile([128, V], F32)
                sE = bp.tile([128, 1], F32)
                nc.scalar.activation(E[:, :], lg_ps[:, :], EXP, accum_out=sE[:, :])
                rE = bp.tile([128, 1], F32)
                nc.vector.reciprocal(rE[:, :], sE[:, :])
                attn = bp.tile([128, V], F32)
                nc.scalar.mul(attn[:, :], E[:, :], rE[:, 0:1])
                for vc in range(4):
                    tp = psT.tile([128, 128], F32)
                    nc.tensor.transpose(tp[:, :], attn[:, ts(vc, 128)], ident[:, :])
                    nc.scalar.copy(attnT[:, vc, ts(cb, 128)], tp[:, :])

        # ---------- phase D: main loop ----------
        stp = ctx.enter_context(tc.tile_pool(name="stem", bufs=3))
        otp = ctx.enter_context(tc.tile_pool(name="outp", bufs=3))
        prp = ctx.enter_context(tc.tile_pool(name="prod", bufs=2))
        psD = ctx.enter_context(tc.tile_pool(name="psD", bufs=2, space="PSUM"))

        PSW = 2048              # one psum tile = 4 banks = 4 matmul groups
        NH = HALF // PSW        # 2 psum tiles per (cb, round)
        # delta = affix - stem is computed eagerly (it does not depend on the
        # wC AllGather), draining PSUM so the PE streams freely. The drain
        # alternates between two paths to balance DVE and ScalarE:
        #   even tiles: ScalarE copy PSUM->bf16, then DVE sub at 2x
        #   odd tiles:  DVE sub straight from PSUM at 1x
        # The wC-gated mul/add then run at DVE 2x (all-SBUF bf16 operands).
        dlp = ctx.enter_context(tc.tile_pool(name="delta", bufs=12))
        rwp = ctx.enter_context(tc.tile_pool(name="draw", bufs=2))

        gi = 0
        for r in range(2):
            for cb in range(BCH):
                stem_t = stp.tile([128, HALF], stem_dt)
                nc.sync.dma_start(
                    stem_t[:, :],
                    stem_d[ts(cb, 128), ts(r, DHALF), :].rearrange("p d n -> p (d n)"),
                )
                for h in range(NH):
                    ps = psD.tile([128, PSW], F32)
                    for t in range(PSW // 512):
                        col = h * PSW + t * 512
                        for vc in range(4):
                            nc.tensor.matmul(
                                ps[:, ts(t, 512)],
                                lhsT=attnT[:, vc:vc + 1, ts(cb, 128)],
                                rhs=vocab_sb[:, vc, r * HALF + col: r * HALF + col + 512],
                                start=(vc == 0), stop=(vc == 3),
                            )
                    delta_t = dlp.tile([128, PSW], BF16)
                    if gi % 2 == 0:
                        raw_t = rwp.tile([128, PSW], BF16)
                        nc.scalar.copy(raw_t[:, :], ps[:, :])
                        nc.vector.tensor_sub(delta_t[:, :], raw_t[:, :],
                                             stem_t[:, ts(h, PSW)])
                    else:
                        nc.vector.tensor_sub(delta_t[:, :], ps[:, :],
                                             stem_t[:, ts(h, PSW)])
                    gi += 1
                    prod = prp.tile([128, PSW], BF16)
                    nc.vector.tensor_mul(
                        prod[:, :].rearrange("p (a n) -> p a n", n=N),
                        delta_t[:, :].rearrange("p (a n) -> p a n", n=N),
                        wc_sb[:, cb:cb + 1, :].to_broadcast((128, PSW // N, N)),
                    )
                    out_t = otp.tile([128, PSW], out_dt)
                    nc.vector.tensor_add(out_t[:, :], prod[:, :],
                                         stem_t[:, ts(h, PSW)])
                    nc.sync.dma_start(
                        out_d[ts(cb, 128), bass.ds(r * DHALF + h * (PSW // N), PSW // N), :]
                        .rearrange("p d n -> p (d n)"),
                        out_t[:, :],
                    )

    nc.compile()
    _CACHE[key] = nc
    return nc


def kernel(stem_form, morphosyn, pivot_logits, W_affix, affix_vocab,
           alpha, beta, phi, max_len):
    global LAST_RESULT
    stem_form = np.ascontiguousarray(np.asarray(stem_form, dtype=np.float32))
    morphosyn = np.ascontiguousarray(np.asarray(morphosyn, dtype=np.float32))
    pivot_logits = np.ascontiguousarray(np.asarray(pivot_logits, dtype=np.float32))
    W_affix = np.ascontiguousarray(np.asarray(W_affix, dtype=np.float32))
    affix_vocab = np.ascontiguousarray(np.asarray(affix_vocab, dtype=np.float32))
    abf = np.concatenate([
        np.asarray(alpha, np.float32).ravel(),
        np.asarray(beta, np.float32).ravel(),
        np.asarray(phi, np.float32).ravel(),
    ]).reshape(1, 9)

    nc = _build()

    stem_np = stem_form.astype(ml_dtypes.bfloat16) if STEM_BF16 else stem_form
    vocab_np = affix_vocab.astype(ml_dtypes.bfloat16) if VOCAB_BF16 else affix_vocab

    in_maps = []
    for c in range(NCORES):
        dlo, dhi = c * DLOC, (c + 1) * DLOC
        in_maps.append({
            "stem": np.ascontiguousarray(stem_np[:, dlo:dhi, :]),
            "vocab": np.ascontiguousarray(vocab_np[:, dlo:dhi, :]),
            "morpho": morphosyn,
            "waffix": W_affix,
            "pivot": np.ascontiguousarray(pivot_logits[:, :, c * 128:(c + 1) * 128, :, :]),
            "abf": abf,
        })

    LAST_RESULT = run_bass_kernel_spmd(nc, in_maps, core_ids=list(range(NCORES)))
    outs = [LAST_RESULT.results[c]["out"] for c in range(NCORES)]
    out = np.concatenate([o.astype(np.float32) for o in outs], axis=1)
    return np.ascontiguousarray(out)


# revision 28
# speedup vs baseline: 1.2446x; 1.0121x over previous
"""Trainium2 Bass kernel for nn_MixtureCogrammar.

Computation (reference):
    attn  = softmax(morphosyn @ W_affix)                    [B, V]
    affix = attn @ affix_vocab.reshape(V, D*N)              [B, D, N]
    wC    = cumsum_n( sum_{ijk} a_i b_j f_k softmax(pivot_logits[i,j,:,k,:]) )
    out   = stem + wC * (affix - stem)

Distribution: the D axis (free dim of the dominant [B,V]x[V,D*N] matmul)
is sharded over the 8 cores (D_local = 32 per core). Every core computes
the full attention (cheap), the pivot/wC path is batch-sharded with an
AllGather, and affix_vocab / stem / out are D-sharded so each core
touches 1/8 of the heavy tensors.

Per-core device program:
  - mixture weights a (x) b (x) f computed on device from alpha/beta/phi
  - attn computed per 128-batch chunk, transposed on the PE into [V, B]
  - big matmul accumulates 4 K-chunks (bf16) plus a (-I) @ stem matmul
    (float32r) into PSUM, so PSUM holds delta = affix - stem
  - DVE blends: out = stem + wC * delta (2 tensor-tensor ops per tile)
"""

import os
import sys

import numpy as np

for _p in ("/opt/trn_rl_repo",):
    if os.path.isdir(_p) and _p not in sys.path:
        sys.path.append(_p)

import concourse.bass as bass  # noqa: E402
import concourse.tile as tile  # noqa: E402
from concourse import bacc, mybir  # noqa: E402
from concourse.bass import ts  # noqa: E402
from concourse.bass_utils import run_bass_kernel_spmd  # noqa: E402
from concourse.masks import make_identity  # noqa: E402

import ml_dtypes  # noqa: E402

B, D, N, DM, V = 1024, 256, 256, 128, 512
NCORES = 8
DLOC = D // NCORES          # 32 d-values per core
BCH = B // 128              # 8 batch chunks
DN = DLOC * N               # 8192 free elems per core
HALF = DN // 2              # 4096 per round
NT = HALF // 512            # 8 psum tiles per (chunk, round)
DHALF = DLOC // 2           # 16 d-values per round

F32 = mybir.dt.float32
F32R = mybir.dt.float32r
BF16 = mybir.dt.bfloat16
EXP = mybir.ActivationFunctionType.Exp
ALU = mybir.AluOpType

# knobs
VOCAB_BF16 = True    # host-cast affix_vocab to bf16 (halves its DMA)
STEM_BF16 = True     # host-cast stem to bf16 (halves stem DMA, adds ~1e-3 err)
OUT_BF16 = True      # write output as bf16, upcast on host

LAST_RESULT = None   # BassKernelResults of the last run (exec_time_ns etc.)

_CACHE = {}


def _build():
    key = (VOCAB_BF16, STEM_BF16, OUT_BF16)
    if key in _CACHE:
        return _CACHE[key]

    stem_dt = BF16 if STEM_BF16 else F32
    vocab_dt = BF16 if VOCAB_BF16 else F32
    out_dt = BF16 if OUT_BF16 else F32

    nc = bacc.Bacc("TRN2", target_bir_lowering=False, debug=False,
                   num_devices=NCORES)

    stem_d = nc.dram_tensor("stem", [B, DLOC, N], stem_dt, kind="ExternalInput").ap()
    vocab_d = nc.dram_tensor("vocab", [V, DLOC, N], vocab_dt, kind="ExternalInput").ap()
    mor_d = nc.dram_tensor("morpho", [B, DM], F32, kind="ExternalInput").ap()
    waff_d = nc.dram_tensor("waffix", [DM, V], F32, kind="ExternalInput").ap()
    pv_d = nc.dram_tensor("pivot", [2, 2, 128, 5, N], F32, kind="ExternalInput").ap()
    abf_d = nc.dram_tensor("abf", [1, 9], F32, kind="ExternalInput").ap()
    out_d = nc.dram_tensor("out", [B, DLOC, N], out_dt, kind="ExternalOutput").ap()

    from contextlib import ExitStack

    with tile.TileContext(nc) as tc, ExitStack() as ctx:
        const = ctx.enter_context(tc.tile_pool(name="const", bufs=1))

        ident = const.tile([128, 128], F32)
        make_identity(nc, ident[:, :])

        attnT = const.tile([128, 4, B], BF16)      # [v_part, vc, b]
        wc_sb = const.tile([128, BCH, N], BF16)    # [b_part, cb, n]
        w_bcast = const.tile([128, 20], F32)
        wsb = const.tile([128, V], F32)            # W_affix resident
        mor_all = const.tile([128, BCH, DM], F32)  # morphosyn, all chunks
        vocab_sb = const.tile([128, 4, DN], vocab_dt)  # [v_part, vc, (d n)]

        # ---------- phase A: mixture weights ----------
        small = ctx.enter_context(tc.tile_pool(name="small", bufs=1))
        # attn pool allocated before the pivot pool so the attention phase's
        # tiles do not reuse (and WAR-serialize behind) the pivot tiles
        bp = ctx.enter_context(tc.tile_pool(name="attn", bufs=2))
        pvp = tc.alloc_tile_pool(name="pv", bufs=1)
        pv = pvp.tile([128, 4, 5, N], F32)
        abf = small.tile([1, 9], F32)
        nc.sync.dma_start(abf[0:1, :], abf_d[:, :])
        for ij in range(4):
            i, j = divmod(ij, 2)
            nc.sync.dma_start(pv[:, ij, :, :], pv_d[i, j, :, :, :])
        nc.sync.dma_start(wsb[:, :], waff_d[:, :])
        for cb in range(BCH):
            nc.sync.dma_start(mor_all[:, cb, :], mor_d[ts(cb, 128), :])
        eabf = small.tile([1, 9], F32)
        sums = small.tile([1, 3], F32)
        nc.scalar.activation(eabf[0:1, 0:2], abf[0:1, 0:2], EXP, accum_out=sums[0:1, 0:1])
        nc.scalar.activation(eabf[0:1, 2:4], abf[0:1, 2:4], EXP, accum_out=sums[0:1, 1:2])
        nc.scalar.activation(eabf[0:1, 4:9], abf[0:1, 4:9], EXP, accum_out=sums[0:1, 2:3])
        rsum = small.tile([1, 3], F32)
        nc.vector.reciprocal(rsum[0:1, :], sums[0:1, :])
        t4 = small.tile([1, 4], F32)
        nc.vector.tensor_mul(
            t4[0:1, :].rearrange("p (i j) -> p i j", i=2),
            eabf[0:1, 0:2].rearrange("p (i j) -> p i j", j=1).to_broadcast((1, 2, 2)),
            eabf[0:1, 2:4].rearrange("p (i j) -> p i j", i=1).to_broadcast((1, 2, 2)),
        )
        t20 = small.tile([1, 20], F32)
        nc.vector.tensor_mul(
            t20[0:1, :].rearrange("p (g k) -> p g k", g=4),
            t4[0:1, :].rearrange("p (g k) -> p g k", k=1).to_broadcast((1, 4, 5)),
            eabf[0:1, 4:9].rearrange("p (g k) -> p g k", g=1).to_broadcast((1, 4, 5)),
        )
        rr = small.tile([1, 1], F32)
        nc.vector.tensor_mul(rr[0:1, :], rsum[0:1, 0:1], rsum[0:1, 1:2])
        rrr = small.tile([1, 1], F32)
        nc.vector.tensor_mul(rrr[0:1, :], rr[0:1, :], rsum[0:1, 2:3])
        w20 = small.tile([1, 20], F32)
        nc.vector.tensor_scalar_mul(w20[0:1, :], t20[0:1, :], rrr[0:1, 0:1])
        nc.gpsimd.partition_broadcast(w_bcast[:, :], w20[0:1, :])

        # ---------- phase C: pivots (this core's batch chunk) ----------
        if True:
            pvE = pvp.tile([128, 20, N], F32)
            sP = pvp.tile([128, 20], F32)
            for g in range(20):
                nc.scalar.activation(pvE[:, g, :], pv[:, g // 5, g % 5, :], EXP)
                nc.vector.reduce_sum(sP[:, g:g + 1], pvE[:, g, :],
                                     axis=mybir.AxisListType.X)
            rP = pvp.tile([128, 20], F32)
            nc.vector.reciprocal(rP[:, :], sP[:, :])
            rPw = pvp.tile([128, 20], F32)
            nc.vector.tensor_mul(rPw[:, :], rP[:, :], w_bcast[:, :])
            accA = pvp.tile([128, N], F32)
            accB = pvp.tile([128, N], F32)
            nc.vector.tensor_scalar_mul(accA[:, :], pvE[:, 0, :], rPw[:, 0:1])
            cur, nxt = accA, accB
            for g in range(1, 20):
                nc.vector.scalar_tensor_tensor(
                    out=nxt[:, :], in0=pvE[:, g, :], scalar=rPw[:, g:g + 1],
                    in1=cur[:, :], op0=ALU.mult, op1=ALU.add,
                )
                cur, nxt = nxt, cur
            wCl = pvp.tile([128, N], BF16)
            nc.vector.tensor_tensor_scan(
                wCl[:, :], data0=cur[:, :], data1=cur[:, :], initial=0.0,
                op0=ALU.add, op1=ALU.bypass,
            )
            dram = ctx.enter_context(tc.tile_pool(name="dram", bufs=1, space="DRAM"))
            wc_in = dram.tile([128, N], BF16)
            wc_out = nc.dram_tensor("wc_gath", [B, N], BF16,
                                    addr_space="Shared").ap()
            nc.sync.dma_start(wc_in[:, :], wCl[:, :])
            nc.gpsimd.collective_compute(
                "AllGather", ALU.bypass,
                replica_groups=[list(range(NCORES))],
                ins=[wc_in[:, :].opt()], outs=[wc_out[:, :].opt()],
            )
            nc.sync.dma_start(
                wc_sb[:, :, :],
                wc_out[:, :].rearrange("(c p) n -> p c n", p=128),
            )

        pvp.release()
        # vocab loads: emitted after the pivot path so the small latency-
        # critical DMAs get queue priority; r0 halves first.
        for r in range(2):
            for vc in range(4):
                nc.sync.dma_start(
                    vocab_sb[:, vc, r * HALF:(r + 1) * HALF],
                    vocab_d[ts(vc, 128), ts(r, DHALF), :].rearrange("p d n -> p (d n)"),
                )

        # ---------- phase B: attention for all batches ----------
        with tc.tile_pool(name="psB", bufs=2, space="PSUM") as psB, \
             tc.tile_pool(name="psT", bufs=2, space="PSUM") as psT:
            for cb in range(BCH):
                morT_ps = psB.tile([128, DM], F32, tag="morT_ps")
                nc.tensor.transpose(morT_ps[:, :], mor_all[:, cb, :], ident[:, :])
                morT = bp.tile([128, DM], F32)
                nc.scalar.copy(morT[:, :], morT_ps[:, :])
                lg_ps = psB.tile([128, V], F32, tag="lg_ps")
                nc.tensor.matmul(lg_ps[:, :], lhsT=morT[:, :], rhs=wsb[:, :],
                                 start=True, stop=True)
                E = bp.tile([128, V], F32)
                sE = bp.tile([128, 1], F32)
                nc.scalar.activation(E[:, :], lg_ps[:, :], EXP, accum_out=sE[:, :])
                rE = bp.tile([128, 1], F32)
                nc.vector.reciprocal(rE[:, :], sE[:, :])
                attn = bp.tile([128, V], F32)
                nc.scalar.mul(attn[:, :], E[:, :], rE[:, 0:1])
                for vc in range(4):
                    tp = psT.tile([128, 128], F32)
                    nc.tensor.transpose(tp[:, :], attn[:, ts(vc, 128)], ident[:, :])
                    nc.scalar.copy(attnT[:, vc, ts(cb, 128)], tp[:, :])

        # ---------- phase D: main loop ----------
        stp = ctx.enter_context(tc.tile_pool(name="stem", bufs=3))
        otp = ctx.enter_context(tc.tile_pool(name="outp", bufs=3))
        prp = ctx.enter_context(tc.tile_pool(name="prod", bufs=2))
        psD = ctx.enter_context(tc.tile_pool(name="psD", bufs=2, space="PSUM"))

        PSW = 2048              # one psum tile = 4 banks = 4 matmul groups
        NH = HALF // PSW        # 2 psum tiles per (cb, round)
        # delta = affix - stem is computed eagerly (it does not depend on the
        # wC AllGather), draining PSUM so the PE streams freely. The drain
        # alternates between two paths to balance DVE and ScalarE:
        #   even tiles: ScalarE copy PSUM->bf16, then DVE sub at 2x
        #   odd tiles:  DVE sub straight from PSUM at 1x
        # The wC-gated mul/add then run at DVE 2x (all-SBUF bf16 operands).
        dlp = ctx.enter_context(tc.tile_pool(name="delta", bufs=11))
        rwp = ctx.enter_context(tc.tile_pool(name="draw", bufs=2))

        gi = 0
        for r in range(2):
            for cb in range(BCH):
                stem_t = stp.tile([128, HALF], stem_dt)
                nc.sync.dma_start(
                    stem_t[:, :],
                    stem_d[ts(cb, 128), ts(r, DHALF), :].rearrange("p d n -> p (d n)"),
                )
                for h in range(NH):
                    ps = psD.tile([128, PSW], F32)
                    for t in range(PSW // 512):
                        col = h * PSW + t * 512
                        for vc in range(4):
                            nc.tensor.matmul(
                                ps[:, ts(t, 512)],
                                lhsT=attnT[:, vc:vc + 1, ts(cb, 128)],
                                rhs=vocab_sb[:, vc, r * HALF + col: r * HALF + col + 512],
                                start=(vc == 0), stop=(vc == 3),
                            )
                    delta_t = dlp.tile([128, PSW], BF16)
                    if gi % 2 == 0:
                        raw_t = rwp.tile([128, PSW], BF16)
                        nc.scalar.copy(raw_t[:, :], ps[:, :])
                        nc.vector.tensor_sub(delta_t[:, :], raw_t[:, :],
                                             stem_t[:, ts(h, PSW)])
                    else:
                        nc.vector.tensor_sub(delta_t[:, :], ps[:, :],
                                             stem_t[:, ts(h, PSW)])
                    gi += 1
                    prod = prp.tile([128, PSW], BF16)
                    nc.vector.tensor_mul(
                        prod[:, :].rearrange("p (a n) -> p a n", n=N),
                        delta_t[:, :].rearrange("p (a n) -> p a n", n=N),
                        wc_sb[:, cb:cb + 1, :].to_broadcast((128, PSW // N, N)),
                    )
                    out_t = otp.tile([128, PSW], out_dt)
                    nc.vector.tensor_add(out_t[:, :], prod[:, :],
                                         stem_t[:, ts(h, PSW)])
                    nc.sync.dma_start(
                        out_d[ts(cb, 128), bass.ds(r * DHALF + h * (PSW // N), PSW // N), :]
                        .rearrange("p d n -> p (d n)"),
                        out_t[:, :],
                    )

    nc.compile()
    _CACHE[key] = nc
    return nc


def kernel(stem_form, morphosyn, pivot_logits, W_affix, affix_vocab,
           alpha, beta, phi, max_len):
    global LAST_RESULT
    stem_form = np.ascontiguousarray(np.asarray(stem_form, dtype=np.float32))
    morphosyn = np.ascontiguousarray(np.asarray(morphosyn, dtype=np.float32))
    pivot_logits = np.ascontiguousarray(np.asarray(pivot_logits, dtype=np.float32))
    W_affix = np.ascontiguousarray(np.asarray(W_affix, dtype=np.float32))
    affix_vocab = np.ascontiguousarray(np.asarray(affix_vocab, dtype=np.float32))
    abf = np.concatenate([
        np.asarray(alpha, np.float32).ravel(),
        np.asarray(beta, np.float32).ravel(),
        np.asarray(phi, np.float32).ravel(),
    ]).reshape(1, 9)

    nc = _build()

    stem_np = stem_form.astype(ml_dtypes.bfloat16) if STEM_BF16 else stem_form
    vocab_np = affix_vocab.astype(ml_dtypes.bfloat16) if VOCAB_BF16 else affix_vocab

    in_maps = []
    for c in range(NCORES):
        dlo, dhi = c * DLOC, (c + 1) * DLOC
        in_maps.append({
            "stem": np.ascontiguousarray(stem_np[:, dlo:dhi, :]),
            "vocab": np.ascontiguousarray(vocab_np[:, dlo:dhi, :]),
            "morpho": morphosyn,
            "waffix": W_affix,
            "pivot": np.ascontiguousarray(pivot_logits[:, :, c * 128:(c + 1) * 128, :, :]),
            "abf": abf,
        })

    LAST_RESULT = run_bass_kernel_spmd(nc, in_maps, core_ids=list(range(NCORES)))
    outs = [LAST_RESULT.results[c]["out"] for c in range(NCORES)]
    out = np.concatenate([o.astype(np.float32) for o in outs], axis=1)
    return np.ascontiguousarray(out)
